# revision 1
# baseline (speedup 1.0000x reference)
"""GTU (Gated Toeplitz Unit) kernel for 8 Trainium2 NeuronCores.

Strategy: data-parallel over batch (B=8 -> 1 batch element per core) via
jax.pmap on the neuron PJRT backend. The reference's FFT-based Toeplitz
mixing is replaced by explicit Toeplitz matrix construction + batched
matmuls (bit-equivalent math, verified against np.fft offline), since
FFT does not lower to trn2 while dense matmuls map to the PE array.

Self-contained: shapes/sharding hardcoded per the problem spec.
B, H, W, E = 8, 128, 128, 192; NH=8, D1=576, HD=72, RPE=512, LAYERS=3.
"""
import numpy as np
import jax
import jax.numpy as jnp

B, HH, WW, E = 8, 128, 128, 192
NH = 8
D1 = 576
HD = D1 // NH  # 72
RPE = 512
LAYERS = 3
EPS = 1e-8
N = 128          # mixing length (H == W == 128)
TWO_N = 2 * N    # 256


def _srms(x):
    d = x.shape[-1]
    norm = jnp.sqrt(jnp.sum(x * x, axis=-1, keepdims=True))
    return x / (norm * (d ** -0.5) + EPS)


def _rpe_net(idx, pos_w, pos_b, lw, lb, out_w, out_b):
    # idx: (2N, 1) -> coefficients (NH, 2N, HD)
    h = idx @ pos_w.T + pos_b
    for i in range(LAYERS):
        h = jax.nn.relu(_srms(h)) @ lw[i].T + lb[i]
    h = jax.nn.relu(_srms(h)) @ out_w.T + out_b  # (2N, NH*HD)
    return h.reshape(TWO_N, NH, HD).transpose(1, 0, 2)  # (NH, 2N, HD)


def _toeplitz_mats(a, gather_idx):
    # a: (NH, 2N, HD) -> T: (NH*HD, N, N) with T[c,i,j] = a[h, (i-j)%2N, d]
    T = a[:, gather_idx, :]            # (NH, N, N, HD)
    T = T.transpose(0, 3, 1, 2)        # (NH, HD, N, N)
    return T.reshape(NH * HD, N, N)


def _gtu_one(x, params):
    # x: (H, W, E) one batch element on one core
    (u_w, u_b, v_w, v_b, o_w, o_b, T1, T2) = params
    bf = jnp.bfloat16
    f32 = jnp.float32
    shortcut = x
    xn = _srms(x).astype(bf)
    u = jax.nn.silu(jnp.matmul(xn, u_w.T.astype(bf),
                               preferred_element_type=f32) + u_b)  # (H, W, D1)
    v = jax.nn.silu(jnp.matmul(xn, v_w.T.astype(bf),
                               preferred_element_type=f32) + v_b)  # (H, W, D1)

    # (H, W, NH, HD) -> (NH*HD, H, W) channel-major for batched matmuls
    vc = v.reshape(HH, WW, NH, HD).transpose(2, 3, 0, 1).reshape(NH * HD, HH, WW)
    vcb = vc.astype(bf)

    # TNO along H: out[c,i,w] = sum_j T2[c,i,j] v[c,j,w]
    oH = jnp.matmul(T2.astype(bf), vcb, preferred_element_type=f32)
    # TNO along W: out[c,h,w] = sum_j T1[c,w,j] v[c,h,j]
    oW = jnp.matmul(vcb, T1.transpose(0, 2, 1).astype(bf),
                    preferred_element_type=f32)
    o = oH + oW

    o = o.reshape(NH, HD, HH, WW).transpose(2, 3, 0, 1).reshape(HH, WW, D1)
    o = (u * o).astype(bf)
    return jnp.matmul(o, o_w.T.astype(bf),
                      preferred_element_type=f32) + o_b + shortcut


def _gtu_sharded(x, u_w, u_b, v_w, v_b, o_w, o_b,
                 r1pw, r1pb, r1lw, r1lb, r1ow, r1ob,
                 r2pw, r2pb, r2lw, r2lb, r2ow, r2ob,
                 idx, gather_idx):
    # Runs per-core under pmap. RPE nets are tiny; computed on every core.
    a1 = _rpe_net(idx, r1pw, r1pb, r1lw, r1lb, r1ow, r1ob)
    a2 = _rpe_net(idx, r2pw, r2pb, r2lw, r2lb, r2ow, r2ob)
    T1 = _toeplitz_mats(a1, gather_idx)
    T2 = _toeplitz_mats(a2, gather_idx)
    return _gtu_one(x, (u_w, u_b, v_w, v_b, o_w, o_b, T1, T2))


_PMAPPED = None


def _get_pmapped():
    global _PMAPPED
    if _PMAPPED is None:
        _PMAPPED = jax.pmap(
            _gtu_sharded,
            in_axes=(0,) + (None,) * 20,
        )
    return _PMAPPED


def kernel(**inputs) -> np.ndarray:
    x = np.asarray(inputs["x"], dtype=np.float32)

    # Toeplitz position index layout of the reference:
    # coefficient vector positions [0, 1..N-1, 0, -(N-1)..-1], and
    # T[i,j] = a[(i-j) mod 2N].
    z = np.zeros((1,), np.float32)
    p = np.arange(1, N, dtype=np.float32)
    idx = np.concatenate([z, p, z, -p[::-1]]).reshape(-1, 1)  # (2N, 1)
    ii = np.arange(N)[:, None]
    jj = np.arange(N)[None, :]
    gather_idx = ((ii - jj) % TWO_N).astype(np.int32)          # (N, N)

    fn = _get_pmapped()
    args = (
        jnp.asarray(x),
        jnp.asarray(inputs["u_w"]), jnp.asarray(inputs["u_b"]),
        jnp.asarray(inputs["v_w"]), jnp.asarray(inputs["v_b"]),
        jnp.asarray(inputs["o_w"]), jnp.asarray(inputs["o_b"]),
        jnp.asarray(inputs["rpe1_pos_w"]), jnp.asarray(inputs["rpe1_pos_b"]),
        jnp.asarray(inputs["rpe1_lw"]), jnp.asarray(inputs["rpe1_lb"]),
        jnp.asarray(inputs["rpe1_out_w"]), jnp.asarray(inputs["rpe1_out_b"]),
        jnp.asarray(inputs["rpe2_pos_w"]), jnp.asarray(inputs["rpe2_pos_b"]),
        jnp.asarray(inputs["rpe2_lw"]), jnp.asarray(inputs["rpe2_lb"]),
        jnp.asarray(inputs["rpe2_out_w"]), jnp.asarray(inputs["rpe2_out_b"]),
        jnp.asarray(idx), jnp.asarray(gather_idx),
    )
    out = fn(*args)
    return np.asarray(out, dtype=np.float32)


if __name__ == "__main__":
    rng = np.random.default_rng(0)
    demo = {
        "x": rng.standard_normal((B, HH, WW, E), dtype=np.float32),
        "u_w": rng.standard_normal((D1, E), dtype=np.float32) * 0.02,
        "u_b": rng.standard_normal((D1,), dtype=np.float32) * 0.02,
        "v_w": rng.standard_normal((D1, E), dtype=np.float32) * 0.02,
        "v_b": rng.standard_normal((D1,), dtype=np.float32) * 0.02,
        "o_w": rng.standard_normal((E, D1), dtype=np.float32) * 0.02,
        "o_b": rng.standard_normal((E,), dtype=np.float32) * 0.02,
    }
    for nm in ("rpe1", "rpe2"):
        demo[nm + "_pos_w"] = rng.standard_normal((RPE, 1), dtype=np.float32) * 0.5
        demo[nm + "_pos_b"] = rng.standard_normal((RPE,), dtype=np.float32) * 0.5
        demo[nm + "_lw"] = rng.standard_normal((LAYERS, RPE, RPE), dtype=np.float32) * 0.02
        demo[nm + "_lb"] = rng.standard_normal((LAYERS, RPE), dtype=np.float32) * 0.02
        demo[nm + "_out_w"] = rng.standard_normal((D1, RPE), dtype=np.float32) * 0.02
        demo[nm + "_out_b"] = rng.standard_normal((D1,), dtype=np.float32) * 0.02
    demo["H"] = HH
    demo["W"] = WW
    y = kernel(**demo)
    print("out", y.shape, y.dtype)

